# revision 8
# baseline (speedup 1.0000x reference)
import numpy as np

N_RAYS = 65536
S = 128
N_CORES = 8
R_CORE = N_RAYS // N_CORES  # 8192
NRAY = 8                    # rays packed per partition row
ROW = NRAY * S              # 1024 floats per partition row
TILE_RAYS = 128 * NRAY      # 1024 rays per tile

_BUILD_CACHE = {}


def _mkap(T, off, dims):
    from concourse.bass_types import AP
    return AP(T.tensor, T.offset + off, [list(T.ap[0])] + [list(d) for d in dims])


def _squeeze(dims):
    d = [x for x in dims if x[1] != 1]
    return d or [[1, 1]]


def _emit_sort(nc, engines, col_splits, bufA, bufB, copy_eng=None):
    """Batcher odd-even mergesort of NRAY independent 128-runs per row.

    All comparators ascending, so every min/max AP has <=3 free dims
    (HW TensorTensor limit). Stages with k<p leave two boundary blocks
    per 2p-group untouched; those are copied on copy_eng.
    Ping-pongs bufA<->bufB; 28 stages (even) so result lands in bufA.
    """
    import concourse.mybir as mybir
    mn, mx = mybir.AluOpType.min, mybir.AluOpType.max
    if copy_eng is None:
        copy_eng = nc.vector
    cur, oth = bufA, bufB
    p = 1
    while p < S:
        k = p
        while k >= 1:
            for eng, (lo, hi) in zip(engines, col_splits):
                w = hi - lo
                nA = w // (2 * p)
                if k == p:
                    dims = _squeeze([[2 * p, nA], [1, p]])
                    src0 = _mkap(cur, lo, dims)
                    src1 = _mkap(cur, lo + p, dims)
                    eng.tensor_tensor(_mkap(oth, lo, dims), src0, src1, op=mn)
                    eng.tensor_tensor(_mkap(oth, lo + p, dims), src0, src1, op=mx)
                else:
                    cnt = p // k - 1
                    dims = _squeeze([[2 * p, nA], [2 * k, cnt], [1, k]])
                    src0 = _mkap(cur, lo + k, dims)
                    src1 = _mkap(cur, lo + 2 * k, dims)
                    eng.tensor_tensor(_mkap(oth, lo + k, dims), src0, src1, op=mn)
                    eng.tensor_tensor(_mkap(oth, lo + 2 * k, dims), src0, src1, op=mx)
            if k < p:
                cdims = _squeeze([[2 * p, ROW // (2 * p)], [2 * p - k, 2], [1, k]])
                copy_eng.tensor_scalar_add(
                    _mkap(oth, 0, cdims), _mkap(cur, 0, cdims), 0.0)
            cur, oth = oth, cur
            k //= 2
        p *= 2
    assert cur is bufA
    return bufA


def build(rpc=R_CORE, sort_pool_cols=0, copy_engine="vector"):
    """Build the Bass program for one core processing rpc rays."""
    import concourse.bass as bass
    import concourse.mybir as mybir
    from concourse import tile

    f32 = mybir.dt.float32
    op = mybir.AluOpType
    ntiles = rpc // TILE_RAYS
    assert rpc % TILE_RAYS == 0

    nc = bass.Bass()
    t_in = nc.dram_tensor("t", [rpc, S], f32, kind="ExternalInput")
    sg_in = nc.dram_tensor("sigma", [rpc, S], f32, kind="ExternalInput")
    c_in = nc.dram_tensor("c", [rpc, 3 * S], f32, kind="ExternalInput")
    wi_out = nc.dram_tensor("wi", [rpc, S], f32, kind="ExternalOutput")
    col_out = nc.dram_tensor("color", [rpc, 3], f32, kind="ExternalOutput")
    dep_out = nc.dram_tensor("depth", [rpc, 1], f32, kind="ExternalOutput")

    if sort_pool_cols > 0:
        engines = [nc.vector, nc.gpsimd]
        splits = [(0, ROW - sort_pool_cols), (ROW - sort_pool_cols, ROW)]
    else:
        engines = [nc.vector]
        splits = [(0, ROW)]

    with tile.TileContext(nc) as tc:
        with tc.tile_pool(name="consts", bufs=1) as cpool, \
             tc.tile_pool(name="main", bufs=3) as pool:
            mask = cpool.tile([128, ROW], f32)
            nc.vector.memset(mask, 1.0)
            nc.vector.memset(_mkap(mask, 0, [[S, NRAY], [1, 1]]), 0.0)

            for i in range(ntiles):
                base = i * TILE_RAYS
                ping = pool.tile([128, ROW], f32, name="ping")
                pong = pool.tile([128, ROW], f32, name="pong")
                sg = pool.tile([128, ROW], f32, name="sg")
                ct = pool.tile([128, 3 * ROW], f32, name="ct")
                gbuf = pool.tile([128, NRAY * (S + 1)], f32, name="gbuf")
                colr = pool.tile([128, NRAY * 3], f32, name="colr")
                dpth = pool.tile([128, NRAY], f32, name="dpth")

                rows = slice(base, base + TILE_RAYS)
                nc.sync.dma_start(
                    out=ping.rearrange("p (r s) -> p r s", r=NRAY),
                    in_=t_in[rows, :].rearrange("(p r) s -> p r s", r=NRAY))
                nc.sync.dma_start(
                    out=sg.rearrange("p (r s) -> p r s", r=NRAY),
                    in_=sg_in[rows, :].rearrange("(p r) s -> p r s", r=NRAY))
                nc.sync.dma_start(
                    out=ct.rearrange("p (r s) -> p r s", r=NRAY),
                    in_=c_in[rows, :].rearrange("(p r) s -> p r s", r=NRAY))

                ts = _emit_sort(nc, engines, splits, ping, pong,
                                copy_eng=getattr(nc, copy_engine))

                # dt -> pong ; dt[:, :, S-1] = 0
                dims127 = [[S, NRAY], [1, S - 1]]
                nc.vector.tensor_tensor(
                    _mkap(pong, 0, dims127), _mkap(ts, 1, dims127),
                    _mkap(ts, 0, dims127), op=op.subtract)
                nc.vector.memset(_mkap(pong, S - 1, [[S, NRAY], [1, 1]]), 0.0)

                # sdt = sigma * dt  (in-place into sg)
                nc.vector.tensor_tensor(sg, sg, pong, op=op.mult)

                # cum = segmented inclusive cumsum of sdt -> pong
                nc.vector.tensor_tensor_scan(
                    out=pong, data0=mask, data1=sg, initial=0.0,
                    op0=op.mult, op1=op.add)

                # G = exp(-cum) into gbuf[:, r, 1:S+1]; gbuf[:, r, 0] = 1
                nc.vector.memset(_mkap(gbuf, 0, [[S + 1, NRAY], [1, 1]]), 1.0)
                nc.scalar.activation(
                    _mkap(gbuf, 1, [[S + 1, NRAY], [1, S]]),
                    _mkap(pong, 0, [[S, NRAY], [1, S]]),
                    func=mybir.ActivationFunctionType.Exp, bias=0.0, scale=-1.0)

                # wi = G[i-1] - G[i]  (into sg, overwriting sdt)
                dimsS = [[S, NRAY], [1, S]]
                nc.vector.tensor_tensor(
                    _mkap(sg, 0, dimsS),
                    _mkap(gbuf, 0, [[S + 1, NRAY], [1, S]]),
                    _mkap(gbuf, 1, [[S + 1, NRAY], [1, S]]), op=op.subtract)
                nc.sync.dma_start(
                    out=wi_out[rows, :].rearrange("(p r) s -> p r s", r=NRAY),
                    in_=sg.rearrange("p (r s) -> p r s", r=NRAY))

                # depth = sum_i wi_i * ts_i
                nc.vector.tensor_tensor(pong, sg, ts, op=op.mult)
                nc.vector.tensor_reduce(
                    out=dpth, in_=pong.rearrange("p (r s) -> p r s", r=NRAY),
                    axis=mybir.AxisListType.X, op=op.add)
                nc.sync.dma_start(
                    out=dep_out[rows, :].rearrange("(p r) one -> p r one", r=NRAY),
                    in_=dpth.rearrange("p (r one) -> p r one", one=1))

                # color = sum_i wi_i * c_i   (cmul in-place into ct)
                ct4 = ct.rearrange("p (r s ch) -> p r s ch", r=NRAY, ch=3)
                wib = _mkap(sg, 0, [[S, NRAY], [1, S], [0, 3]])
                nc.vector.tensor_tensor(ct4, ct4, wib, op=op.mult)
                nc.vector.tensor_reduce(
                    out=colr.rearrange("p (r ch) -> p r ch", ch=3),
                    in_=_mkap(ct, 0, [[3 * S, NRAY], [1, 3], [3, S]]),
                    axis=mybir.AxisListType.X, op=op.add)
                nc.sync.dma_start(
                    out=col_out[rows, :].rearrange("(p r) ch -> p r ch", r=NRAY),
                    in_=colr.rearrange("p (r ch) -> p r ch", ch=3))
    return nc


def _legalize_waits(bj):
    """Split multi-sem waits: this walrus build allows one wait per
    instruction, so hoist extras onto same-engine NoOps just before."""
    import json
    d = json.loads(bj)
    ctr = 0
    for f in d["functions"]:
        for b in f["blocks"]:
            out = []
            for ins in b["instructions"]:
                si = ins.get("sync_info")
                waits = (si or {}).get("on_wait") or []
                if len(waits) > 1:
                    for w in waits[:-1]:
                        ctr += 1
                        out.append({
                            "debug": ins.get("debug", 0),
                            "engine": ins.get("engine"),
                            "ins": [], "outs": [],
                            "name": f"I-lw-{ctr}",
                            "opcode": "NoOp",
                            "sync_info": {"on_update": [], "on_wait": [w]},
                        })
                    si["on_wait"] = [waits[-1]]
                out.append(ins)
            b["instructions"] = out
    return json.dumps(d).encode()


def _finalize(nc):
    bj = _legalize_waits(nc.to_json_bytes())
    nc.to_json_bytes = lambda: bj
    return nc


def _get_nc():
    key = (R_CORE,)
    if key not in _BUILD_CACHE:
        _BUILD_CACHE[key] = _finalize(build(R_CORE))
    return _BUILD_CACHE[key]


def kernel(t, sigma, c):
    from concourse.bass_utils import run_bass_kernel_spmd
    t2 = np.ascontiguousarray(t.reshape(N_RAYS, S).astype(np.float32))
    s2 = np.ascontiguousarray(sigma.reshape(N_RAYS, S).astype(np.float32))
    c2 = np.ascontiguousarray(c.reshape(N_RAYS, 3 * S).astype(np.float32))
    nc = _get_nc()
    in_maps = [
        {"t": t2[k * R_CORE:(k + 1) * R_CORE],
         "sigma": s2[k * R_CORE:(k + 1) * R_CORE],
         "c": c2[k * R_CORE:(k + 1) * R_CORE]}
        for k in range(N_CORES)
    ]
    res = run_bass_kernel_spmd(nc, in_maps, list(range(N_CORES)))
    color = np.concatenate([res.results[k]["color"] for k in range(N_CORES)], 0)
    depth = np.concatenate([res.results[k]["depth"] for k in range(N_CORES)], 0)
    wi = np.concatenate([res.results[k]["wi"] for k in range(N_CORES)], 0)
    return color, depth, wi.reshape(N_RAYS, S, 1)
